# revision 36
# baseline (speedup 1.0000x reference)
"""Multi-head causal attention on 8 Trainium2 cores (v2).

Sharding: 8 cores = 4 batches x 2 head-groups (8 heads each); host sums the
two head-group partials per batch (the "all-reduce") and pre-transposes
x/pos/W per shard (pure layout prep) so the device never runs PE transposes.

Per-core dataflow (bf16 matmul operands, fp32 PSUM):
  W:     DMA wT [m, ih] f32 -> cast bf16; woT [h-pair, chunk, m] likewise.
  B(sb): DMA xT/posT [m, 512-seq] f32 tiles -> DVE add -> xqT bf16 (+ cast
         xT bf16); QT/KT [ih-pair, chunk, seq] accumulate over 8 m-chunks;
         V [seq, i*(h|1)+pad] with a ones column (softmax normalizer free).
  C(c,qb): per key tile: scoresT [k, 2-head, q] via row-paired (tile_position)
         matmuls, diagonal tiles column-trimmed and causal-masked by
         accumulating a -1e5 staircase through an ident @ M matmul; one ACT
         exp (scale=1/8) covers both heads; zps[hh] += V_kt.T @ ex.
  norm:  z+Z rows copied out of PSUM, Z DMA'd to partition 0, gpsimd
         partition-broadcast, reciprocal_approx_fast, DVE mults -> zTf bf16;
         odd head-half placed via SBUF->SBUF DMA partition shift.
  D(qb): out[q, m] accumulates zTf.T @ woT over 4 chunks -> DMA out.
Emission interleaves B(sb+1) load/proj half-chain units and deferred D-wave
units into C(qb)'s key-tile loop so the PE stays dense while ACT chews the
exps; bulk input DMAs round-robin over the sync/scalar/gpsimd queues (a
single queue sustains only ~220-270 GB/s), with each staging buffer pinned
to one queue (cross-queue writes to a rotating buffer race).
"""

import sys

if "/opt/trn_rl_repo" not in sys.path:
    sys.path.insert(0, "/opt/trn_rl_repo")

import numpy as np

SEQ = 2048
DM = 1024
NH = 8           # heads per core
DH = 64
IH = NH * DH     # 512
MC = DM // 128   # 8 m-chunks
ST = SEQ // 128  # 16 seq tiles
NQB = SEQ // 512  # 4 query blocks
NC_CH = NH // 2  # 4 head-pair chunks
MVAL = -100000.0
# Schraudolph-style exp for bf16 via int16 bit trick (DVE overflow valve for
# ACT): bits16(exp(0.125*s)) ~= round(EXP_A*s + EXP_B); max rel err ~3.3%.
# Only safe when EXP_A*s+EXP_B stays in int16 range: |s| < ~700, true for
# unmasked (off-diagonal) score tiles.
EXP_A = 128 * 0.125 * 1.4426950408889634
EXP_B = 16250.5

_BUILT = None


def _build():
    import concourse.mybir as mybir
    import concourse.tile as tile
    from concourse import bacc
    from concourse.masks import make_identity

    dt = mybir.dt
    f32, bf16 = dt.float32, dt.bfloat16
    AF = mybir.ActivationFunctionType
    Alu = mybir.AluOpType

    nc = bacc.Bacc("TRN2", target_bir_lowering=False, debug=False)
    xT_d = nc.dram_tensor("xT_s", [DM, SEQ], f32, kind="ExternalInput")
    posT_d = nc.dram_tensor("posT_s", [DM, SEQ], f32, kind="ExternalInput")
    wqT_d = nc.dram_tensor("wqT_s", [DM, IH], f32, kind="ExternalInput")
    wkT_d = nc.dram_tensor("wkT_s", [DM, IH], f32, kind="ExternalInput")
    wvT_d = nc.dram_tensor("wvT_s", [DM, IH], f32, kind="ExternalInput")
    woT_d = nc.dram_tensor("woT_s", [128, NC_CH, DM], f32, kind="ExternalInput")
    out_d = nc.dram_tensor("out_s", [SEQ, DM], f32, kind="ExternalOutput")

    with tile.TileContext(nc) as tc:
        with tc.tile_pool(name="const", bufs=1) as cp, \
             tc.tile_pool(name="big", bufs=1) as bigp, \
             tc.tile_pool(name="wts", bufs=1) as wp, \
             tc.tile_pool(name="xblk", bufs=1) as xblk, \
             tc.tile_pool(name="xstg", bufs=4) as xstg, \
             tc.tile_pool(name="expp", bufs=4) as expp, \
             tc.tile_pool(name="norm", bufs=1) as npl, \
             tc.tile_pool(name="outsb", bufs=2) as outsb, \
             tc.tile_pool(name="mm", bufs=2, space="PSUM") as mmp, \
             tc.tile_pool(name="sc", bufs=2, space="PSUM") as scp, \
             tc.tile_pool(name="zp", bufs=1, space="PSUM") as zpp:

            # ---------------- constants -------------------------------
            identb = cp.tile([128, 128], bf16)
            make_identity(nc, identb[:])
            maskb = cp.tile([128, 128], bf16)  # M[r,c] = 0 if c>=r else MVAL
            nc.gpsimd.memset(maskb[:], 0.0)
            nc.gpsimd.affine_select(
                out=maskb[:], in_=maskb[:], compare_op=Alu.is_ge,
                fill=MVAL, base=0, pattern=[[1, 128]], channel_multiplier=-1)
            ones_st = cp.tile([128, 1], f32)
            nc.gpsimd.memset(ones_st[:], 1.0)
            zero_st = cp.tile([128, 1], f32)
            nc.gpsimd.memset(zero_st[:], 0.0)

            # ---------------- persistent SBUF tensors -----------------
            QT = bigp.tile([128, NC_CH, SEQ], bf16)   # [pair-dim, chunk, seq]
            KT = bigp.tile([128, NC_CH, SEQ], bf16)
            V = bigp.tile([128, ST, NH * (DH + 1) + 63], bf16)
            zTf = bigp.tile([128, NC_CH, SEQ], bf16)  # [pair-dim, chunk, q]
            wqT = wp.tile([128, MC, IH], bf16)        # [m-in, m-chunk, ih]
            wkT = wp.tile([128, MC, IH], bf16)
            wvT = wp.tile([128, MC, IH], bf16)
            woT = wp.tile([128, NC_CH, DM], bf16)     # [pair-dim, chunk, m]

            # zero V's pad + ones column
            nc.vector.tensor_copy(
                V[:, :, NH * (DH + 1):],
                zero_st[:, 0:1].to_broadcast([128, ST, 63]))
            nc.vector.tensor_copy(
                V[:, :, 0:NH * (DH + 1)].rearrange(
                    "p s (i x) -> p s i x", i=NH)[:, :, :, DH:DH + 1],
                ones_st[:, 0:1].to_broadcast([128, ST, NH, 1]))

            # ---------------- weight loads + casts --------------------
            # DMA queue spreading: all of sync/scalar (HWDGE) and gpsimd
            # (SWDGE) have their own hardware DMA queue; a single queue
            # sustains only ~220-270 GB/s, so the input loads round-robin
            # across all three to approach the HBM limit.
            with tc.tile_pool(name="wstg", bufs=2) as wstg:
                def w_chunk_units(w_d, wT, qoff=0):
                    """Per-m-chunk DMA + cast units for one [DM, IH] weight.

                    Each DMA queue gets its own staging tag: a pool buffer
                    must only ever be DMA-written from one queue (same-engine
                    DMAs are FIFO-ordered, cross-queue writes race).
                    """
                    engs = [nc.sync, nc.scalar]
                    units = []
                    for mc in range(MC):
                        def u(mc=mc):
                            q = (mc + qoff) % 2
                            ws = wstg.tile([128, IH], f32, tag=f"w{q}",
                                           name="ws")
                            engs[q].dma_start(
                                ws[:],
                                w_d.ap()[mc * 128:(mc + 1) * 128, :])
                            nc.vector.tensor_copy(wT[:, mc, :], ws[:])
                        units.append(u)
                    return units

                def wo_units():
                    units = []
                    for c in range(NC_CH):
                        def u(c=c):
                            ws = wstg.tile([128, DM], f32, tag="wo",
                                           name="wos")
                            nc.scalar.dma_start(ws[:], woT_d.ap()[:, c, :])
                            nc.vector.tensor_copy(woT[:, c, :], ws[:])
                        units.append(u)
                    return units

                # ---------------- work-unit machinery ---------------------
                def b_load_units(sb):
                    """DMA xT/posT m-chunk tiles, add -> xqT bf16, cast xT."""
                    xqTb = xblk.tile([128, MC, 512], bf16, tag=f"xq{sb % 2}",
                                     name=f"xqTb{sb}")
                    xTb = xblk.tile([128, MC, 512], bf16, tag=f"xt{sb % 2}",
                                    name=f"xTb{sb}")
                    units = []
                    for mc in range(MC):
                        def u(mc=mc, xqTb=xqTb, xTb=xTb):
                            xs = xstg.tile([128, 512], f32, tag="x", name="xs")
                            nc.scalar.dma_start(
                                xs[:], xT_d.ap()[mc * 128:(mc + 1) * 128,
                                                 sb * 512:(sb + 1) * 512])
                            ps_ = xstg.tile([128, 512], f32, tag="pos",
                                            name="ps")
                            nc.sync.dma_start(
                                ps_[:], posT_d.ap()[mc * 128:(mc + 1) * 128,
                                                    sb * 512:(sb + 1) * 512])
                            nc.vector.tensor_add(xqTb[:, mc, :], xs[:], ps_[:])
                            nc.vector.tensor_copy(xTb[:, mc, :], xs[:])
                        units.append(u)
                    return (xqTb, xTb), units

                def qk_proj_units(sb, blks, wT, dstT):
                    """Each chunk's 8-matmul accumulation split into two
                    half-chain units so interleave pacing is ~0.9 us grained.
                    """
                    xqTb, _ = blks
                    units = []
                    for c in range(NC_CH):
                        hold = {}
                        def uA(c=c, hold=hold):
                            ps = mmp.tile([128, 512], f32, tag="mm",
                                          name="ps_qk")
                            hold["ps"] = ps
                            for mc in range(4):
                                nc.tensor.matmul(
                                    ps[:],
                                    wT[:, mc, c * 128:(c + 1) * 128],
                                    xqTb[:, mc, :],
                                    start=(mc == 0), stop=False)
                        def uB(c=c, hold=hold):
                            ps = hold["ps"]
                            for mc in range(4, MC):
                                nc.tensor.matmul(
                                    ps[:],
                                    wT[:, mc, c * 128:(c + 1) * 128],
                                    xqTb[:, mc, :],
                                    start=False, stop=(mc == MC - 1))
                            nc.vector.tensor_copy(
                                dstT[:, c, sb * 512:(sb + 1) * 512], ps[:])
                        units += [uA, uB]
                    return units

                def v_proj_units(sb, blks):
                    _, xTb = blks
                    units = []
                    for stl in range(4):
                        st = sb * 4 + stl
                        hold = {}
                        def uA(stl=stl, hold=hold):
                            ps = mmp.tile([128, 512], f32, tag="mm",
                                          name="ps_v")
                            hold["ps"] = ps
                            for mc in range(4):
                                nc.tensor.matmul(
                                    ps[:],
                                    xTb[:, mc, stl * 128:(stl + 1) * 128],
                                    wvT[:, mc, :],
                                    start=(mc == 0), stop=False)
                        def uB(st=st, stl=stl, hold=hold):
                            ps = hold["ps"]
                            for mc in range(4, MC):
                                nc.tensor.matmul(
                                    ps[:],
                                    xTb[:, mc, stl * 128:(stl + 1) * 128],
                                    wvT[:, mc, :],
                                    start=False, stop=(mc == MC - 1))
                            nc.vector.tensor_copy(
                                V[:, st, 0:NH * (DH + 1)].rearrange(
                                    "p (i x) -> p i x", i=NH)[:, :, 0:DH],
                                ps[:].rearrange("p (i h) -> p i h", i=NH))
                        units += [uA, uB]
                    return units

                def b_proj_units(sb, blks):
                    return (qk_proj_units(sb, blks, wqT, QT)
                            + qk_proj_units(sb, blks, wkT, KT)
                            + v_proj_units(sb, blks))

                def d_units(qb):
                    units = []
                    for qtl in range(4):
                        qt = qb * 4 + qtl
                        osb = outsb.tile([128, DM], f32, tag="osb",
                                         name=f"osb{qt}")
                        for mb in range(2):
                            hold = {}
                            def uA(qt=qt, mb=mb, hold=hold):
                                po = mmp.tile([128, 512], f32, tag="mm",
                                              name="po")
                                hold["po"] = po
                                for c in range(2):
                                    nc.tensor.matmul(
                                        po[:],
                                        zTf[:, c, qt * 128:(qt + 1) * 128],
                                        woT[:, c, mb * 512:(mb + 1) * 512],
                                        start=(c == 0), stop=False)
                            def uB(qt=qt, mb=mb, osb=osb, hold=hold):
                                po = hold["po"]
                                for c in range(2, NC_CH):
                                    nc.tensor.matmul(
                                        po[:],
                                        zTf[:, c, qt * 128:(qt + 1) * 128],
                                        woT[:, c, mb * 512:(mb + 1) * 512],
                                        start=False, stop=(c == NC_CH - 1))
                                nc.vector.tensor_copy(
                                    osb[:, mb * 512:(mb + 1) * 512], po[:])
                                nc.gpsimd.dma_start(
                                    out_d.ap()[qt * 128:(qt + 1) * 128,
                                               mb * 512:(mb + 1) * 512],
                                    osb[:, mb * 512:(mb + 1) * 512])
                            units += [uA, uB]
                    return units

                def emit_c(c, qb, zps):
                    nkt = 4 * qb + 4
                    for kt in range(nkt):
                        j = kt - 4 * qb
                        diag = j >= 0
                        off = 128 * j if diag else 0
                        sc = scp.tile([128, 2, 512], f32, tag="sc", name="sc")
                        for hh in range(2):
                            r0 = hh * 64
                            nc.tensor.matmul(
                                sc[:, hh, off:512],
                                KT[r0:r0 + 64, c, kt * 128:(kt + 1) * 128],
                                QT[r0:r0 + 64, c,
                                   qb * 512 + off:(qb + 1) * 512],
                                start=True, stop=not diag,
                                tile_position=(r0, 0))
                        if diag:
                            for hh in range(2):
                                nc.tensor.matmul(
                                    sc[:, hh, off:off + 128],
                                    identb[:], maskb[:],
                                    start=False, stop=True)
                        ex = expp.tile([128, 2, 512], bf16, tag="ex",
                                       name="ex")
                        nc.scalar.activation(ex[:, :, off:512],
                                             sc[:, :, off:512],
                                             AF.Exp, scale=0.125)
                        for hh in range(2):
                            i = 2 * c + hh
                            nc.tensor.matmul(
                                zps[hh][:, off:512],
                                V[:, kt, i * (DH + 1):i * (DH + 1) + 128],
                                ex[:, hh, off:512],
                                start=(kt == 0), stop=(kt == nkt - 1))
                        yield

                def emit_norm(c, qb, zps):
                    t0 = npl.tile([65, 512], f32, tag="t0", name="t0")
                    t1 = npl.tile([65, 512], f32, tag="t1", name="t1")
                    nc.vector.tensor_copy(t0[:], zps[0][0:65, :])
                    nc.vector.tensor_copy(t1[:], zps[1][0:65, :])
                    zr0 = npl.tile([1, 512], f32, tag="zr0", name="zr0")
                    zr1 = npl.tile([1, 512], f32, tag="zr1", name="zr1")
                    nc.sync.dma_start(zr0[:], t0[64:65, :])
                    nc.sync.dma_start(zr1[:], t1[64:65, :])
                    ri0 = npl.tile([1, 512], f32, tag="ri0", name="ri0")
                    ri1 = npl.tile([1, 512], f32, tag="ri1", name="ri1")
                    nc.vector.reciprocal_approx_fast(out=ri0[:], in_=zr0[:])
                    nc.vector.reciprocal_approx_fast(out=ri1[:], in_=zr1[:])
                    bc0 = npl.tile([64, 512], f32, tag="bc0", name="bc0")
                    bc1 = npl.tile([64, 512], f32, tag="bc1", name="bc1")
                    nc.gpsimd.partition_broadcast(bc0[:], ri0[:])
                    nc.gpsimd.partition_broadcast(bc1[:], ri1[:])
                    nc.vector.tensor_mul(
                        zTf[0:64, c, qb * 512:(qb + 1) * 512],
                        t0[0:64, :], bc0[:])
                    stg = npl.tile([64, 512], bf16, tag="stg", name="stg")
                    nc.vector.tensor_mul(stg[:], t1[0:64, :], bc1[:])
                    nc.sync.dma_start(
                        zTf[64:128, c, qb * 512:(qb + 1) * 512], stg[:])

                # ---------------- main schedule ---------------------------
                # Startup: interleave wq chunks with x/pos(sb0) loads so the
                # QT projections can start as soon as ~6 MB have landed; wk
                # and wv stream in under the QT/KT matmuls; wo defers to
                # wave 0's unit list (first needed by D(0) in wave 1).
                blks = {}
                blks[0], lu0 = b_load_units(0)
                wq_u = w_chunk_units(wqT_d, wqT, qoff=0)
                wk_u = w_chunk_units(wkT_d, wkT, qoff=1)
                xqTb0, _ = blks[0]
                # QT c0/c1 chains advance one matmul per landed m-chunk so
                # the PE tracks the DMA stream through the load-bound start
                # instead of stalling past HAM windows (which kept all of
                # B(0) at the 1.2 GHz cold clock). Uses only the two mm-pool
                # buffers; the rest of B(0) runs dense once data is resident.
                am = [mmp.tile([128, 512], f32, tag="mm", name=f"acc{j}")
                      for j in range(2)]
                for mc in range(MC):
                    wq_u[mc]()
                    wk_u[mc]()
                    lu0[mc]()
                    for j in range(2):
                        nc.tensor.matmul(
                            am[j],
                            wqT[:, mc, j * 128:(j + 1) * 128],
                            xqTb0[:, mc, :],
                            start=(mc == 0), stop=(mc == MC - 1))
                for j in range(2):
                    nc.vector.tensor_copy(QT[:, j, 0:512], am[j])
                for u in qk_proj_units(0, blks[0], wqT, QT)[4:]:
                    u()
                for u in w_chunk_units(wvT_d, wvT, qoff=0):
                    u()
                for u in qk_proj_units(0, blks[0], wkT, KT):
                    u()
                for u in v_proj_units(0, blks[0]):
                    u()

                # Unit assignment per wave, balanced against each wave's PE
                # slack (wave 3 has the most ACT-bound attention work, so the
                # deferrable D waves land there): w0: wo+B(1); w1: B(2)+D(0);
                # w2: B(3); w3: D(1)+D(2); tail: D(3).
                for qb in range(NQB):
                    units = []
                    if qb == 0:
                        units += wo_units()
                    if qb + 1 < NQB:
                        blks[qb + 1], lu = b_load_units(qb + 1)
                        units += lu
                        units += b_proj_units(qb + 1, blks[qb + 1])
                    if qb == 1:
                        units += d_units(0)
                    elif qb == 3:
                        units += d_units(1) + d_units(2)
                    total_kts = NC_CH * (4 * qb + 4)
                    nkt = 4 * qb + 4
                    done = 0
                    emitted = 0
                    for c in range(NC_CH):
                        zps = [zpp.tile([128, 512], f32, tag=f"z{hh}",
                                        name=f"z{hh}") for hh in range(2)]
                        kt_in_c = 0
                        for _ in emit_c(c, qb, zps):
                            done += 1
                            kt_in_c += 1
                            if kt_in_c >= nkt:
                                break  # norm first; catch up after
                            target = (len(units) * done) // total_kts
                            while emitted < target:
                                units[emitted]()
                                emitted += 1
                        emit_norm(c, qb, zps)
                        target = (len(units) * done) // total_kts
                        while emitted < target:
                            units[emitted]()
                            emitted += 1
                    while emitted < len(units):
                        units[emitted]()
                        emitted += 1
                for u in d_units(NQB - 1):
                    u()

    nc.compile()
    return nc


def _get_nc():
    global _BUILT
    if _BUILT is None:
        _BUILT = _build()
    return _BUILT


def _prep_core(x_b, pos_b, wq_g, wk_g, wv_g, wo_g):
    woT = np.empty((128, NC_CH, DM), dtype=np.float32)
    for c in range(NC_CH):
        for hh in range(2):
            woT[hh * 64:(hh + 1) * 64, c, :] = wo_g[2 * c + hh].T
    return {
        "xT_s": np.ascontiguousarray(x_b.T),
        "posT_s": np.ascontiguousarray(pos_b.T),
        "wqT_s": np.ascontiguousarray(wq_g.reshape(IH, DM).T),
        "wkT_s": np.ascontiguousarray(wk_g.reshape(IH, DM).T),
        "wvT_s": np.ascontiguousarray(wv_g.reshape(IH, DM).T),
        "woT_s": woT,
    }


def run(inputs, trace=False):
    from concourse import bass_utils

    nc = _get_nc()
    x = np.asarray(inputs["x"], dtype=np.float32)
    pos = np.asarray(inputs["pos_embed"], dtype=np.float32)
    wq, wk, wv, wo = (np.asarray(inputs[k], dtype=np.float32)
                      for k in ("W_Q", "W_K", "W_V", "W_O"))
    in_maps = []
    for core in range(8):
        b, g = core // 2, core % 2
        hs = slice(g * NH, (g + 1) * NH)
        in_maps.append(_prep_core(x[b], pos[b], wq[hs], wk[hs], wv[hs],
                                  wo[hs]))
    res = bass_utils.run_bass_kernel_spmd(
        nc, in_maps, core_ids=list(range(8)), trace=trace)
    out = np.empty((4, SEQ, DM), dtype=np.float32)
    for b in range(4):
        out[b] = res.results[2 * b]["out_s"] + res.results[2 * b + 1]["out_s"]
    return out, res.exec_time_ns


def kernel(**inputs):
    out, _ = run(inputs, trace=False)
    return out
